# revision 12
# baseline (speedup 1.0000x reference)
"""Trainium2 Bass kernel for nn_DeepFeatureLoss (pairwise softmax-correspondence loss).

Math (per batch b):
    P = softmax_j(-||x_i - x_j||^2 / s^2),   s = SIGMA
    F = softmax_j(-||f1_i - f2_j||^2)
    out[b] = sum_i w_i * sum_j (P_ij - F_ij)^2

Strategy: shard rows i across 8 cores (512 rows each). Host precomputes
fp16 hi/lo compensated-product operands for BOTH score matmuls (3 stacked
blocks along the contraction axis -> hi.hi + hi.lo + lo.hi with fp32 PSUM
accumulation, ~22-bit effective mantissa):
    spatial: K = 3*5  = 15   (3 coords + 2 norm rows per block)
    feature: K = 3*34 = 102  (32 coords + 2 norm rows per block)
Scores stream through ScalarE Exp (per-row bias -|x_i|^2/s^2 resp. -|f1_i|^2,
so values are <= ~1) into bf16 fields e1,e2 (bf16 keeps fp32-like range: rows
whose nearest feature neighbour is ~16+ away have e2 ~ 1e-15 which underflows
fp16 and would zero the softmax denominator).

ScalarE does ONLY the 32 exp ACTIVATEs (no accumulator reads). Row sums
s1,s2 come from DVE tensor_scalar at 4x rate (16-bit single-src); then
    d = (s1/s2)*e2 - e1        (scalar_tensor_tensor, 2x rate, in place)
    qv = sum_j d^2             (scalar_tensor_tensor d*d with accum_out)
    loss rows = qv / s1^2, weighted and reduced per batch.
Per-core partial losses [128 lanes, B] are summed on host.
"""

import os
import sys

import numpy as np

sys.path.insert(0, "/opt/trn_rl_repo")

import concourse.bass as bass
import concourse.tile as tile
from concourse import mybir
from concourse.bass_utils import run_bass_kernel_spmd

# If the environment sets BASS_TRACE, run_bass_kernel_spmd imports
# antenv.axon_hooks; provide a null-hook fallback when the image lacks it.
try:
    import antenv.axon_hooks  # noqa: F401
except Exception:
    try:
        import types

        import antenv

        _m = types.ModuleType("antenv.axon_hooks")
        _m._hook = None
        _m.set_axon_ntff_profile_hook = lambda h: setattr(_m, "_hook", h)
        _m.get_axon_ntff_profile_hook = lambda: _m._hook
        sys.modules["antenv.axon_hooks"] = _m
        antenv.axon_hooks = _m
    except Exception:
        pass

SIGMA = 0.05
B = 2
N = 4096
D = 32
NCORES = 8
RPC = N // NCORES          # rows per core = 512
TILES = RPC // 128         # i-tiles per core per batch = 4
KS = 15                    # spatial contraction: 3 blocks of (3 coords + 2 norm)
KF = 3 * (D + 2)           # feature contraction: 3 blocks of (32 coords + 2 norm)
KFH = KF // 2              # feature rows are split 51+51 at SBUF partitions 0/64
                           # so the two halves run on disjoint PE row-groups

FP = mybir.dt.float32
F16 = mybir.dt.float16
BF = mybir.dt.bfloat16
AX = mybir.AxisListType
OP = mybir.AluOpType
AF = mybir.ActivationFunctionType

LAST_RESULT = None         # test harness introspection


def _fix_walrus_incompat(nc):
    """This container's walrus codegen fits exactly ONE sync-wait per engine
    instruction struct (Tile's scheduler freely emits several) and rejects the
    EVENT_SEMAPHORE_RANGE_CLEAR raw-ISA instruction Tile emits at context
    exit. Rewrite: (a) every multi-wait instruction becomes (n-1) same-engine
    EventSemaphore waits followed by the instruction with the final wait;
    (b) the range-clear becomes one sem-wr-imm(0) EventSemaphore per sem."""
    import re

    from bass_rust import SyncInfo, SyncUpdate

    fn = nc.m.functions[0]
    originals = [(blk, list(blk.instructions)) for blk in fn.blocks]
    rebuilt = []
    for blk, insts in originals:
        out = []
        for inst in insts:
            tname = type(inst).__name__
            si = inst.sync_info
            if tname == "InstISA" and "EVENT_SEMAPHORE_RANGE_CLEAR" in inst.concise():
                m = re.search(r"range_first=(\d+) range_last=(\d+)", inst.concise())
                first, last = int(m.group(1)), int(m.group(2))
                for sem in range(first, last + 1):
                    ev = mybir.InstEventSemaphore(
                        name=nc.get_next_instruction_name(),
                        engine=inst.engine,
                        sync_info=SyncInfo(
                            on_wait=list(si.on_wait) if si and sem == first else [],
                            on_update=[
                                SyncUpdate(
                                    sync_type="semaphore",
                                    id=sem,
                                    ant_name=f"semclear_{sem}",
                                    update_mode="sem-wr-imm",
                                    update_value=0,
                                    update_reg=None,
                                )
                            ],
                        ),
                    )
                    nc.register_instruction(ev, overwrite=True)
                    out.append(ev)
                continue
            if si is not None and len(si.on_wait) > 1:
                waits = list(si.on_wait)
                for w in waits[:-1]:
                    ev = mybir.InstEventSemaphore(
                        name=nc.get_next_instruction_name(),
                        engine=inst.engine,
                        sync_info=SyncInfo(on_wait=[w], on_update=[]),
                    )
                    nc.register_instruction(ev, overwrite=True)
                    out.append(ev)
                inst.sync_info = SyncInfo(
                    on_wait=[waits[-1]], on_update=list(si.on_update)
                )
            out.append(inst)
        rebuilt.append((blk, out))
    for blk, out in rebuilt:
        blk.instructions[:] = out


def _build_nc(rowsplit=True, ts4=True):
    nc = bass.Bass()

    # lhsT block (cols 0:RPC) FIRST, then rhs (cols RPC:RPC+N) so the first
    # LDWEIGHTS only waits for a small leading DMA chunk.
    spat_comb = nc.dram_tensor("spat_comb", [B, KS, RPC + N], F16, kind="ExternalInput")
    feat_comb = nc.dram_tensor("feat_comb", [B, KF, RPC + N], F16, kind="ExternalInput")
    # biases + weights packed partition-major: smalls[p, tensor*B*TILES + b*TILES + t]
    # = value for row t*128+p of batch b.
    smalls = nc.dram_tensor("smalls", [128, 3 * B * TILES], FP, kind="ExternalInput")
    out = nc.dram_tensor("out", [128, B], FP, kind="ExternalOutput")

    with tile.TileContext(nc) as tc:
        with (
            tc.tile_pool(name="const", bufs=1) as cpool,
            tc.tile_pool(name="psum", bufs=2, space="PSUM") as ppool,
            tc.tile_pool(name="ebuf", bufs=5) as epool,
            tc.tile_pool(name="junk", bufs=4) as jpool,
            tc.tile_pool(name="small", bufs=8) as spool,
            tc.tile_pool(name="spart", bufs=8) as sppool,
            tc.tile_pool(name="accs", bufs=1) as apool,
        ):
            # --- constants / inputs ---
            sm = cpool.tile([128, 3 * B * TILES], FP, tag="smalls")
            bx = [sm[:, b * TILES : (b + 1) * TILES] for b in range(B)]
            bf = [sm[:, (B + b) * TILES : (B + b + 1) * TILES] for b in range(B)]
            wt = [sm[:, (2 * B + b) * TILES : (2 * B + b + 1) * TILES] for b in range(B)]

            # dummy 8-col exp: forces the ACT exp table load during DMA wait
            dum = cpool.tile([128, 8], FP, tag="dum")
            nc.gpsimd.memset(dum[:], 0.0)
            dj = cpool.tile([128, 8], FP, tag="dj")
            nc.scalar.activation(dj[:], dum[:], AF.Exp)

            # whole-tensor transfers: contiguous DRAM rows pack into few
            # large descriptors (column-sliced 2D DMAs cost ~1 descriptor per
            # partition row and take 2-3us just to issue)
            # issue order: f0 (big transfer overlaps the spatial compute),
            # smalls, s0, f1, s1
            scomb, fcomb = [], []
            for b in range(B):
                ft = cpool.tile([128, RPC + N], F16, tag=f"f{b}")
                if rowsplit:
                    for h, p0 in ((0, 0), (1, 64)):
                        nc.sync.dma_start(
                            ft[p0 : p0 + KFH, :],
                            feat_comb[b][h * KFH : (h + 1) * KFH, :],
                        )
                else:
                    nc.sync.dma_start(ft[0:KF, :], feat_comb[b][:, :])
                if b == 0:
                    nc.sync.dma_start(sm[:], smalls[:])
                st = cpool.tile([KS, RPC + N], F16, tag=f"s{b}")
                nc.sync.dma_start(st[:], spat_comb[b])
                scomb.append(st)
                fcomb.append(ft)

            # PE warmup during the DMA fill (HAM p-state nudge; harmless if cold)
            n_warm = int(os.environ.get("DFL_WARMUP", "0"))
            if n_warm:
                wsrc = cpool.tile([128, 512], BF, tag="warm")
                nc.gpsimd.memset(wsrc[:], 1.0)
                for _ in range(n_warm):
                    pw = ppool.tile([128, 2048], FP, tag="ps")
                    nc.tensor.matmul(
                        pw[:, 0:512], wsrc[:, 0:128], wsrc[:], start=True, stop=True
                    )

            outsb = apool.tile([128, B], FP, tag="outsb")

            for b in range(B):
                qv = apool.tile([128, TILES], FP, tag=f"qv{b}")
                r1 = apool.tile([128, TILES], FP, tag=f"r1{b}")
                for t in range(TILES):
                    e1 = epool.tile([128, N], BF, tag="e1")
                    e2 = epool.tile([128, N], BF, tag="e2")
                    spart = sppool.tile([128, 4], FP, tag="spart")
                    for half, (ct, bias_t, ebuf) in enumerate(
                        (
                            (scomb[b], bx[b], e1),
                            (scomb[b], bx[b], e1),
                            (fcomb[b], bf[b], e2),
                            (fcomb[b], bf[b], e2),
                        )
                    ):
                        col0 = (half % 2) * 2048
                        is_feat = half >= 2
                        ps = ppool.tile([128, 2048], FP, tag="ps")
                        for k in range(4):
                            c0 = RPC + col0 + k * 512
                            pseg = ps[:, k * 512 : (k + 1) * 512]
                            if is_feat and rowsplit:
                                # two K=51 halves on disjoint PE row-groups,
                                # accumulated in PSUM: LDW of one half overlaps
                                # the other half's matmul
                                for h, p0 in ((0, 0), (1, 64)):
                                    nc.tensor.matmul(
                                        pseg,
                                        ct[p0 : p0 + KFH, t * 128 : (t + 1) * 128],
                                        ct[p0 : p0 + KFH, c0 : c0 + 512],
                                        start=(h == 0),
                                        stop=(h == 1),
                                    )
                            elif is_feat:
                                nc.tensor.matmul(
                                    pseg,
                                    ct[0:KF, t * 128 : (t + 1) * 128],
                                    ct[0:KF, c0 : c0 + 512],
                                    start=True,
                                    stop=True,
                                )
                            else:
                                nc.tensor.matmul(
                                    pseg,
                                    ct[:, t * 128 : (t + 1) * 128],
                                    ct[:, c0 : c0 + 512],
                                    start=True,
                                    stop=True,
                                )
                        nc.scalar.activation(
                            ebuf[:, col0 : col0 + 2048],
                            ps[:],
                            AF.Exp,
                            bias=bias_t[:, t : t + 1],
                            accum_out=spart[:, half : half + 1],
                        )
                    # sums of the two halves per field, then the scale/sub/square
                    # chain: plain tensor_scalar (4x) + tensor_tensor (2x) + one
                    # 1x stt with accumulator
                    ss = spool.tile([128, 2], FP, tag="ss")
                    nc.vector.tensor_reduce(
                        ss[:],
                        spart[:].rearrange("p (m c) -> p m c", c=2),
                        axis=AX.X,
                        op=OP.add,
                    )
                    nc.vector.reciprocal(r1[:, t : t + 1], ss[:, 0:1])
                    rr2 = spool.tile([128, 1], FP, tag="rr2")
                    nc.vector.reciprocal(rr2[:], ss[:, 1:2])
                    cc = spool.tile([128, 1], FP, tag="cc")
                    nc.vector.tensor_tensor(cc[:], ss[:, 0:1], rr2[:], op=OP.mult)
                    junk = jpool.tile([128, N], BF, tag="junk")
                    nc.vector.tensor_scalar(junk[:], e2[:], cc[:], None, op0=OP.mult)
                    nc.vector.tensor_tensor(e2[:], junk[:], e1[:], op=OP.subtract)
                    nc.vector.scalar_tensor_tensor(
                        junk[:],
                        e2[:],
                        1.0,
                        e2[:],
                        op0=OP.mult,
                        op1=OP.mult,
                        accum_out=qv[:, t : t + 1],
                    )
                # loss rows = qv / s1^2, weighted, reduced over the 4 i-tiles
                i2 = spool.tile([128, TILES], FP, tag="i2")
                nc.vector.tensor_tensor(i2[:], r1[:], r1[:], op=OP.mult)
                q2 = spool.tile([128, TILES], FP, tag="q2")
                nc.vector.tensor_tensor(q2[:], qv[:], i2[:], op=OP.mult)
                lw = spool.tile([128, TILES], FP, tag="lw")
                nc.vector.tensor_tensor(lw[:], q2[:], wt[b], op=OP.mult)
                nc.vector.tensor_reduce(
                    outsb[:, b : b + 1], lw[:], axis=AX.X, op=OP.add
                )
                nc.sync.dma_start(out[:, b : b + 1], outsb[:, b : b + 1])

    _fix_walrus_incompat(nc)
    return nc


_NC_CACHE = {}


def _get_nc():
    rowsplit = os.environ.get("DFL_ROWSPLIT", "") == "1"
    ts4 = os.environ.get("DFL_NO_TS4", "") != "1"
    key = (rowsplit, ts4)
    if key not in _NC_CACHE:
        _NC_CACHE[key] = _build_nc(rowsplit, ts4)
    return _NC_CACHE[key]


def _hi_lo_blocks(y, nrm, x, krows):
    """Stacked compensated-product operands.

    y: [B, C, N] rhs coord rows (full precision), nrm: [B, N] rhs norm row,
    x: [B, C, RPC] lhsT coord rows. krows = C + 2 block height.
    Returns (rhs [B, 3*krows, N], lhsT [B, 3*krows, RPC]) fp16 so that
    lhsT.T @ rhs = x.T @ y + nrm  with ~22-bit effective mantissa.
    """
    nb = y.shape[0]
    ncols = y.shape[2]
    nlhs = x.shape[2]
    C = y.shape[1]
    yh = y.astype(np.float16)
    yl = (y - yh.astype(np.float64)).astype(np.float16)
    nh = nrm.astype(np.float16)
    nl = (nrm - nh.astype(np.float64)).astype(np.float16)
    n2 = (nrm - nh.astype(np.float64) - nl.astype(np.float64)).astype(np.float16)
    xh = x.astype(np.float16)
    xl = (x - xh.astype(np.float64)).astype(np.float16)

    hi_r = np.zeros((nb, krows, ncols), np.float16)
    lo_r = np.zeros((nb, krows, ncols), np.float16)
    hi_l = np.zeros((nb, krows, nlhs), np.float16)
    lo_l = np.zeros((nb, krows, nlhs), np.float16)
    hi_r[:, :C] = yh
    hi_r[:, C] = nh
    hi_r[:, C + 1] = n2
    lo_r[:, :C] = yl
    lo_r[:, C] = nl
    hi_l[:, :C] = xh
    hi_l[:, C] = 1.0
    lo_l[:, :C] = xl
    lo_l[:, C + 1] = 1.0
    rhs = np.concatenate([hi_r, lo_r, hi_r], axis=1)
    lhsT = np.concatenate([hi_l, hi_l, lo_l], axis=1)
    return rhs, lhsT


def _prep_inputs(points, pointfea1, pointfea2, weights):
    """Host-side sharding + operand layout. Returns per-core input maps."""
    s2inv = np.float64(1.0) / (SIGMA * SIGMA)
    x = points.astype(np.float64)        # [B, N, 3]
    f1 = pointfea1.astype(np.float64)    # [B, N, D]
    f2 = pointfea2.astype(np.float64)
    w = weights.astype(np.float32)

    xT = np.swapaxes(x, 1, 2)            # [B, 3, N]
    f1T = np.swapaxes(f1, 1, 2)          # [B, D, N]
    f2T = np.swapaxes(f2, 1, 2)

    xn = np.sum(x * x, axis=2)           # [B, N]
    f1n = np.sum(f1 * f1, axis=2)
    f2n = np.sum(f2 * f2, axis=2)

    # rhs operands are shared across cores; build once
    spat_rhs_all = None
    feat_rhs_all = None

    in_maps = []
    for c in range(NCORES):
        sl = slice(c * RPC, (c + 1) * RPC)
        srhs, slhs = _hi_lo_blocks(
            2.0 * s2inv * xT, -s2inv * xn, xT[:, :, sl], 5
        )
        frhs, flhs = _hi_lo_blocks(2.0 * f2T, -f2n, f1T[:, :, sl], D + 2)
        if spat_rhs_all is None:
            spat_rhs_all = srhs
            feat_rhs_all = frhs
        spat_comb = np.concatenate([slhs, spat_rhs_all], axis=2)
        feat_comb = np.concatenate([flhs, feat_rhs_all], axis=2)

        smalls = np.empty((128, 3 * B * TILES), np.float32)
        for b in range(B):
            bxv = (-s2inv * xn[b, sl]).astype(np.float32).reshape(TILES, 128)
            bfv = (-f1n[b, sl]).astype(np.float32).reshape(TILES, 128)
            wv = w[b, sl].reshape(TILES, 128)
            smalls[:, b * TILES : (b + 1) * TILES] = bxv.T
            smalls[:, (B + b) * TILES : (B + b + 1) * TILES] = bfv.T
            smalls[:, (2 * B + b) * TILES : (2 * B + b + 1) * TILES] = wv.T
        in_maps.append(
            {"spat_comb": spat_comb, "feat_comb": feat_comb, "smalls": smalls}
        )
    return in_maps


def kernel(points, pointfea1, pointfea2, weights):
    global LAST_RESULT
    nc = _get_nc()
    in_maps = _prep_inputs(points, pointfea1, pointfea2, weights)
    res = run_bass_kernel_spmd(nc, in_maps, core_ids=list(range(NCORES)))
    LAST_RESULT = res
    total = np.zeros(B, np.float64)
    for m in res.results:
        total += m["out"].astype(np.float64).sum(axis=0)
    return total.astype(np.float32)


# revision 14
# speedup vs baseline: 1.0262x; 1.0262x over previous
"""Trainium2 Bass kernel for nn_DeepFeatureLoss (pairwise softmax-correspondence loss).

Math (per batch b):
    P = softmax_j(-||x_i - x_j||^2 / s^2),   s = SIGMA
    F = softmax_j(-||f1_i - f2_j||^2)
    out[b] = sum_i w_i * sum_j (P_ij - F_ij)^2

Strategy: shard rows i across 8 cores (512 rows each). Host precomputes
fp16 hi/lo compensated-product operands for BOTH score matmuls (3 stacked
blocks along the contraction axis -> hi.hi + hi.lo + lo.hi with fp32 PSUM
accumulation, ~22-bit effective mantissa):
    spatial: K = 3*5  = 15   (3 coords + 2 norm rows per block)
    feature: K = 3*34 = 102  (32 coords + 2 norm rows per block)
Scores stream through ScalarE Exp (per-row bias -|x_i|^2/s^2 resp. -|f1_i|^2,
so values are <= ~1) into bf16 fields e1,e2 (bf16 keeps fp32-like range: rows
whose nearest feature neighbour is ~16+ away have e2 ~ 1e-15 which underflows
fp16 and would zero the softmax denominator).

Engine balance (all three compute engines land at ~75-85% of the span):
  ScalarE: 32 exp ACTIVATEs of 2048 PSUM cols (~1.9us each, the 1x-rate
    floor) + per-half row-sum accumulators (ACTIVATION_READ_ACCUMULATOR;
    any DVE-side reduction runs at 1x and costs far more).
  VectorE: per unit, junk = (s1/s2)*e2 via tensor_scalar (4x: 16-bit
    single-src), d = junk - e1 via tensor_tensor (2x: 16-bit two-src),
    qv = sum_j d^2 via one 1x scalar_tensor_tensor with accum_out
    (reduce-capable DVE uops only exist at 1x). loss rows = qv / s1^2.
  TensorE: 512-col matmuls (PSUM one-bank limit), fp16 operands stream
    1 col/cycle; inputs land as whole-tensor DMAs (contiguous DRAM rows
    pack into large descriptors; column-sliced 2D DMAs cost ~1
    descriptor per partition row and 2-3us per issue).
Per-core partial losses [128 lanes, B] are summed on host.

Notes from tuning on this container (see git history of the session):
  - DVE perf modes measured: tensor_scalar 4x (1286ns/4096 bf16 cols),
    tensor_tensor 2x (2290ns), ANY accumulating/reduce op 1x (4424ns).
  - Row-split K=51+51 feature matmuls at base partitions 0/64 (PSUM
    accumulation groups across tile positions) compile + pass CoreSim
    but fault the device -> DFL_ROWSPLIT stays off.
  - This walrus cannot compile custom-DVE ops ("ISA wrong length"), so
    the fused (c*e2-e1)^2-accumulate op is not available.
  - ~8us NEFF preamble (engine table loads + entry barrier) and ~7us
    postamble (token-passing barrier + 256-semaphore clear) are fixed
    runtime costs inside the measured exec time.
"""

import os
import sys

import numpy as np

sys.path.insert(0, "/opt/trn_rl_repo")

import concourse.bass as bass
import concourse.tile as tile
from concourse import mybir
from concourse.bass_utils import run_bass_kernel_spmd

# If the environment sets BASS_TRACE, run_bass_kernel_spmd imports
# antenv.axon_hooks; provide a null-hook fallback when the image lacks it.
try:
    import antenv.axon_hooks  # noqa: F401
except Exception:
    try:
        import types

        import antenv

        _m = types.ModuleType("antenv.axon_hooks")
        _m._hook = None
        _m.set_axon_ntff_profile_hook = lambda h: setattr(_m, "_hook", h)
        _m.get_axon_ntff_profile_hook = lambda: _m._hook
        sys.modules["antenv.axon_hooks"] = _m
        antenv.axon_hooks = _m
    except Exception:
        pass

SIGMA = 0.05
B = 2
N = 4096
D = 32
NCORES = 8
RPC = N // NCORES          # rows per core = 512
TILES = RPC // 128         # i-tiles per core per batch = 4
KS = 15                    # spatial contraction: 3 blocks of (3 coords + 2 norm)
KF = 3 * (D + 2)           # feature contraction: 3 blocks of (32 coords + 2 norm)
KFH = KF // 2              # feature rows are split 51+51 at SBUF partitions 0/64
                           # so the two halves run on disjoint PE row-groups

FP = mybir.dt.float32
F16 = mybir.dt.float16
BF = mybir.dt.bfloat16
AX = mybir.AxisListType
OP = mybir.AluOpType
AF = mybir.ActivationFunctionType

LAST_RESULT = None         # test harness introspection


def _fix_walrus_incompat(nc):
    """This container's walrus codegen fits exactly ONE sync-wait per engine
    instruction struct (Tile's scheduler freely emits several) and rejects the
    EVENT_SEMAPHORE_RANGE_CLEAR raw-ISA instruction Tile emits at context
    exit. Rewrite: (a) every multi-wait instruction becomes (n-1) same-engine
    EventSemaphore waits followed by the instruction with the final wait;
    (b) the range-clear becomes one sem-wr-imm(0) EventSemaphore per sem."""
    import re

    from bass_rust import SyncInfo, SyncUpdate

    fn = nc.m.functions[0]
    originals = [(blk, list(blk.instructions)) for blk in fn.blocks]
    rebuilt = []
    for blk, insts in originals:
        out = []
        for inst in insts:
            tname = type(inst).__name__
            si = inst.sync_info
            if tname == "InstISA" and "EVENT_SEMAPHORE_RANGE_CLEAR" in inst.concise():
                m = re.search(r"range_first=(\d+) range_last=(\d+)", inst.concise())
                first, last = int(m.group(1)), int(m.group(2))
                for sem in range(first, last + 1):
                    ev = mybir.InstEventSemaphore(
                        name=nc.get_next_instruction_name(),
                        engine=inst.engine,
                        sync_info=SyncInfo(
                            on_wait=list(si.on_wait) if si and sem == first else [],
                            on_update=[
                                SyncUpdate(
                                    sync_type="semaphore",
                                    id=sem,
                                    ant_name=f"semclear_{sem}",
                                    update_mode="sem-wr-imm",
                                    update_value=0,
                                    update_reg=None,
                                )
                            ],
                        ),
                    )
                    nc.register_instruction(ev, overwrite=True)
                    out.append(ev)
                continue
            if si is not None and len(si.on_wait) > 1:
                waits = list(si.on_wait)
                for w in waits[:-1]:
                    ev = mybir.InstEventSemaphore(
                        name=nc.get_next_instruction_name(),
                        engine=inst.engine,
                        sync_info=SyncInfo(on_wait=[w], on_update=[]),
                    )
                    nc.register_instruction(ev, overwrite=True)
                    out.append(ev)
                inst.sync_info = SyncInfo(
                    on_wait=[waits[-1]], on_update=list(si.on_update)
                )
            out.append(inst)
        rebuilt.append((blk, out))
    for blk, out in rebuilt:
        blk.instructions[:] = out


def _build_nc(rowsplit=True, ts4=True):
    nc = bass.Bass()

    # lhsT block (cols 0:RPC) FIRST, then rhs (cols RPC:RPC+N) so the first
    # LDWEIGHTS only waits for a small leading DMA chunk.
    spat_comb = nc.dram_tensor("spat_comb", [B, KS, RPC + N], F16, kind="ExternalInput")
    feat_comb = nc.dram_tensor("feat_comb", [B, KF, RPC + N], F16, kind="ExternalInput")
    # biases + weights packed partition-major: smalls[p, tensor*B*TILES + b*TILES + t]
    # = value for row t*128+p of batch b.
    smalls = nc.dram_tensor("smalls", [128, 3 * B * TILES], FP, kind="ExternalInput")
    out = nc.dram_tensor("out", [128, B], FP, kind="ExternalOutput")

    with tile.TileContext(nc) as tc:
        with (
            tc.tile_pool(name="const", bufs=1) as cpool,
            tc.tile_pool(name="psum", bufs=2, space="PSUM") as ppool,
            tc.tile_pool(name="ebuf", bufs=4) as epool,
            tc.tile_pool(name="junk", bufs=3) as jpool,
            tc.tile_pool(name="small", bufs=8) as spool,
            tc.tile_pool(name="spart", bufs=8) as sppool,
            tc.tile_pool(name="accs", bufs=1) as apool,
        ):
            # --- constants / inputs ---
            sm = cpool.tile([128, 3 * B * TILES], FP, tag="smalls")
            nc.sync.dma_start(sm[:], smalls[:])
            bx = [sm[:, b * TILES : (b + 1) * TILES] for b in range(B)]
            bf = [sm[:, (B + b) * TILES : (B + b + 1) * TILES] for b in range(B)]
            wt = [sm[:, (2 * B + b) * TILES : (2 * B + b + 1) * TILES] for b in range(B)]

            # dummy 8-col exp: forces the ACT exp table load during DMA wait
            dum = cpool.tile([128, 8], FP, tag="dum")
            nc.gpsimd.memset(dum[:], 0.0)
            dj = cpool.tile([128, 8], FP, tag="dj")
            nc.scalar.activation(dj[:], dum[:], AF.Exp)

            # whole-tensor transfers: contiguous DRAM rows pack into few
            # large descriptors (column-sliced 2D DMAs cost ~1 descriptor per
            # partition row and take 2-3us just to issue)
            scomb, fcomb = [], []
            for b in range(B):
                st = cpool.tile([KS, RPC + N], F16, tag=f"s{b}")
                nc.sync.dma_start(st[:], spat_comb[b])
                ft = cpool.tile([128, RPC + N], F16, tag=f"f{b}")
                if rowsplit:
                    for h, p0 in ((0, 0), (1, 64)):
                        nc.sync.dma_start(
                            ft[p0 : p0 + KFH, :],
                            feat_comb[b][h * KFH : (h + 1) * KFH, :],
                        )
                else:
                    nc.sync.dma_start(ft[0:KF, :], feat_comb[b][:, :])
                scomb.append(st)
                fcomb.append(ft)

            # PE warmup during the DMA fill (HAM p-state nudge; harmless if cold)
            n_warm = int(os.environ.get("DFL_WARMUP", "0"))
            if n_warm:
                wsrc = cpool.tile([128, 512], BF, tag="warm")
                nc.gpsimd.memset(wsrc[:], 1.0)
                for _ in range(n_warm):
                    pw = ppool.tile([128, 2048], FP, tag="ps")
                    nc.tensor.matmul(
                        pw[:, 0:512], wsrc[:, 0:128], wsrc[:], start=True, stop=True
                    )

            outsb = apool.tile([128, B], FP, tag="outsb")

            for b in range(B):
                qv = apool.tile([128, TILES], FP, tag=f"qv{b}")
                r1 = apool.tile([128, TILES], FP, tag=f"r1{b}")
                for t in range(TILES):
                    e1 = epool.tile([128, N], BF, tag="e1")
                    e2 = epool.tile([128, N], BF, tag="e2")
                    spart = sppool.tile([128, 4], FP, tag="spart")
                    for half, (ct, bias_t, ebuf) in enumerate(
                        (
                            (scomb[b], bx[b], e1),
                            (scomb[b], bx[b], e1),
                            (fcomb[b], bf[b], e2),
                            (fcomb[b], bf[b], e2),
                        )
                    ):
                        col0 = (half % 2) * 2048
                        is_feat = half >= 2
                        ps = ppool.tile([128, 2048], FP, tag="ps")
                        for k in range(4):
                            c0 = RPC + col0 + k * 512
                            pseg = ps[:, k * 512 : (k + 1) * 512]
                            if is_feat and rowsplit:
                                # two K=51 halves on disjoint PE row-groups,
                                # accumulated in PSUM: LDW of one half overlaps
                                # the other half's matmul
                                for h, p0 in ((0, 0), (1, 64)):
                                    nc.tensor.matmul(
                                        pseg,
                                        ct[p0 : p0 + KFH, t * 128 : (t + 1) * 128],
                                        ct[p0 : p0 + KFH, c0 : c0 + 512],
                                        start=(h == 0),
                                        stop=(h == 1),
                                    )
                            elif is_feat:
                                nc.tensor.matmul(
                                    pseg,
                                    ct[0:KF, t * 128 : (t + 1) * 128],
                                    ct[0:KF, c0 : c0 + 512],
                                    start=True,
                                    stop=True,
                                )
                            else:
                                nc.tensor.matmul(
                                    pseg,
                                    ct[:, t * 128 : (t + 1) * 128],
                                    ct[:, c0 : c0 + 512],
                                    start=True,
                                    stop=True,
                                )
                        nc.scalar.activation(
                            ebuf[:, col0 : col0 + 2048],
                            ps[:],
                            AF.Exp,
                            bias=bias_t[:, t : t + 1],
                            accum_out=spart[:, half : half + 1],
                        )
                    # sums of the two halves per field, then the scale/sub/square
                    # chain: plain tensor_scalar (4x) + tensor_tensor (2x) + one
                    # 1x stt with accumulator
                    ss = spool.tile([128, 2], FP, tag="ss")
                    nc.vector.tensor_reduce(
                        ss[:],
                        spart[:].rearrange("p (m c) -> p m c", c=2),
                        axis=AX.X,
                        op=OP.add,
                    )
                    nc.vector.reciprocal(r1[:, t : t + 1], ss[:, 0:1])
                    rr2 = spool.tile([128, 1], FP, tag="rr2")
                    nc.vector.reciprocal(rr2[:], ss[:, 1:2])
                    cc = spool.tile([128, 1], FP, tag="cc")
                    nc.vector.tensor_tensor(cc[:], ss[:, 0:1], rr2[:], op=OP.mult)
                    junk = jpool.tile([128, N], BF, tag="junk")
                    nc.vector.tensor_scalar(junk[:], e2[:], cc[:], None, op0=OP.mult)
                    nc.vector.tensor_tensor(e2[:], junk[:], e1[:], op=OP.subtract)
                    nc.vector.scalar_tensor_tensor(
                        junk[:],
                        e2[:],
                        1.0,
                        e2[:],
                        op0=OP.mult,
                        op1=OP.mult,
                        accum_out=qv[:, t : t + 1],
                    )
                # loss rows = qv / s1^2, weighted, reduced over the 4 i-tiles
                i2 = spool.tile([128, TILES], FP, tag="i2")
                nc.vector.tensor_tensor(i2[:], r1[:], r1[:], op=OP.mult)
                q2 = spool.tile([128, TILES], FP, tag="q2")
                nc.vector.tensor_tensor(q2[:], qv[:], i2[:], op=OP.mult)
                lw = spool.tile([128, TILES], FP, tag="lw")
                nc.vector.tensor_tensor(lw[:], q2[:], wt[b], op=OP.mult)
                nc.vector.tensor_reduce(
                    outsb[:, b : b + 1], lw[:], axis=AX.X, op=OP.add
                )
                nc.sync.dma_start(out[:, b : b + 1], outsb[:, b : b + 1])

    _fix_walrus_incompat(nc)
    return nc


_NC_CACHE = {}


def _get_nc():
    rowsplit = os.environ.get("DFL_ROWSPLIT", "") == "1"
    ts4 = os.environ.get("DFL_NO_TS4", "") != "1"
    key = (rowsplit, ts4)
    if key not in _NC_CACHE:
        _NC_CACHE[key] = _build_nc(rowsplit, ts4)
    return _NC_CACHE[key]


def _hi_lo_blocks(y, nrm, x, krows):
    """Stacked compensated-product operands.

    y: [B, C, N] rhs coord rows (full precision), nrm: [B, N] rhs norm row,
    x: [B, C, RPC] lhsT coord rows. krows = C + 2 block height.
    Returns (rhs [B, 3*krows, N], lhsT [B, 3*krows, RPC]) fp16 so that
    lhsT.T @ rhs = x.T @ y + nrm  with ~22-bit effective mantissa.
    """
    nb = y.shape[0]
    ncols = y.shape[2]
    nlhs = x.shape[2]
    C = y.shape[1]
    yh = y.astype(np.float16)
    yl = (y - yh.astype(np.float64)).astype(np.float16)
    nh = nrm.astype(np.float16)
    nl = (nrm - nh.astype(np.float64)).astype(np.float16)
    n2 = (nrm - nh.astype(np.float64) - nl.astype(np.float64)).astype(np.float16)
    xh = x.astype(np.float16)
    xl = (x - xh.astype(np.float64)).astype(np.float16)

    hi_r = np.zeros((nb, krows, ncols), np.float16)
    lo_r = np.zeros((nb, krows, ncols), np.float16)
    hi_l = np.zeros((nb, krows, nlhs), np.float16)
    lo_l = np.zeros((nb, krows, nlhs), np.float16)
    hi_r[:, :C] = yh
    hi_r[:, C] = nh
    hi_r[:, C + 1] = n2
    lo_r[:, :C] = yl
    lo_r[:, C] = nl
    hi_l[:, :C] = xh
    hi_l[:, C] = 1.0
    lo_l[:, :C] = xl
    lo_l[:, C + 1] = 1.0
    rhs = np.concatenate([hi_r, lo_r, hi_r], axis=1)
    lhsT = np.concatenate([hi_l, hi_l, lo_l], axis=1)
    return rhs, lhsT


def _prep_inputs(points, pointfea1, pointfea2, weights):
    """Host-side sharding + operand layout. Returns per-core input maps."""
    s2inv = np.float64(1.0) / (SIGMA * SIGMA)
    x = points.astype(np.float64)        # [B, N, 3]
    f1 = pointfea1.astype(np.float64)    # [B, N, D]
    f2 = pointfea2.astype(np.float64)
    w = weights.astype(np.float32)

    xT = np.swapaxes(x, 1, 2)            # [B, 3, N]
    f1T = np.swapaxes(f1, 1, 2)          # [B, D, N]
    f2T = np.swapaxes(f2, 1, 2)

    xn = np.sum(x * x, axis=2)           # [B, N]
    f1n = np.sum(f1 * f1, axis=2)
    f2n = np.sum(f2 * f2, axis=2)

    # rhs operands are shared across cores; build once
    spat_rhs_all = None
    feat_rhs_all = None

    in_maps = []
    for c in range(NCORES):
        sl = slice(c * RPC, (c + 1) * RPC)
        srhs, slhs = _hi_lo_blocks(
            2.0 * s2inv * xT, -s2inv * xn, xT[:, :, sl], 5
        )
        frhs, flhs = _hi_lo_blocks(2.0 * f2T, -f2n, f1T[:, :, sl], D + 2)
        if spat_rhs_all is None:
            spat_rhs_all = srhs
            feat_rhs_all = frhs
        spat_comb = np.concatenate([slhs, spat_rhs_all], axis=2)
        feat_comb = np.concatenate([flhs, feat_rhs_all], axis=2)

        smalls = np.empty((128, 3 * B * TILES), np.float32)
        for b in range(B):
            bxv = (-s2inv * xn[b, sl]).astype(np.float32).reshape(TILES, 128)
            bfv = (-f1n[b, sl]).astype(np.float32).reshape(TILES, 128)
            wv = w[b, sl].reshape(TILES, 128)
            smalls[:, b * TILES : (b + 1) * TILES] = bxv.T
            smalls[:, (B + b) * TILES : (B + b + 1) * TILES] = bfv.T
            smalls[:, (2 * B + b) * TILES : (2 * B + b + 1) * TILES] = wv.T
        in_maps.append(
            {"spat_comb": spat_comb, "feat_comb": feat_comb, "smalls": smalls}
        )
    return in_maps


def kernel(points, pointfea1, pointfea2, weights):
    global LAST_RESULT
    nc = _get_nc()
    in_maps = _prep_inputs(points, pointfea1, pointfea2, weights)
    res = run_bass_kernel_spmd(nc, in_maps, core_ids=list(range(NCORES)))
    LAST_RESULT = res
    total = np.zeros(B, np.float64)
    for m in res.results:
        total += m["out"].astype(np.float64).sum(axis=0)
    return total.astype(np.float32)


# revision 20
# speedup vs baseline: 1.0357x; 1.0093x over previous
"""Trainium2 Bass kernel for nn_DeepFeatureLoss (pairwise softmax-correspondence loss).

Math (per batch b):
    P = softmax_j(-||x_i - x_j||^2 / s^2),   s = SIGMA
    F = softmax_j(-||f1_i - f2_j||^2)
    out[b] = sum_i w_i * sum_j (P_ij - F_ij)^2

Strategy: shard rows i across 8 cores (512 rows each). Host precomputes
fp16 hi/lo compensated-product operands for BOTH score matmuls (3 stacked
blocks along the contraction axis -> hi.hi + hi.lo + lo.hi with fp32 PSUM
accumulation, ~22-bit effective mantissa):
    spatial: K = 3*5  = 15   (3 coords + 2 norm rows per block)
    feature: K = 3*34 = 102  (32 coords + 2 norm rows per block)
Scores stream through ScalarE Exp (per-row bias -|x_i|^2/s^2 resp. -|f1_i|^2,
so values are <= ~1) into bf16 fields e1,e2 (bf16 keeps fp32-like range: rows
whose nearest feature neighbour is ~16+ away have e2 ~ 1e-15 which underflows
fp16 and would zero the softmax denominator).

ScalarE does ONLY the 32 exp ACTIVATEs (no accumulator reads). Row sums
s1,s2 come from DVE tensor_scalar at 4x rate (16-bit single-src); then
    d = (s1/s2)*e2 - e1        (scalar_tensor_tensor, 2x rate, in place)
    qv = sum_j d^2             (scalar_tensor_tensor d*d with accum_out)
    loss rows = qv / s1^2, weighted and reduced per batch.
Per-core partial losses [128 lanes, B] are summed on host.
"""

import os
import sys

import numpy as np

sys.path.insert(0, "/opt/trn_rl_repo")

import concourse.bass as bass
import concourse.tile as tile
from concourse import mybir
from concourse.bass_utils import run_bass_kernel_spmd

# If the environment sets BASS_TRACE, run_bass_kernel_spmd imports
# antenv.axon_hooks; provide a null-hook fallback when the image lacks it.
try:
    import antenv.axon_hooks  # noqa: F401
except Exception:
    try:
        import types

        import antenv

        _m = types.ModuleType("antenv.axon_hooks")
        _m._hook = None
        _m.set_axon_ntff_profile_hook = lambda h: setattr(_m, "_hook", h)
        _m.get_axon_ntff_profile_hook = lambda: _m._hook
        sys.modules["antenv.axon_hooks"] = _m
        antenv.axon_hooks = _m
    except Exception:
        pass

SIGMA = 0.05
B = 2
N = 4096
D = 32
NCORES = 8
RPC = N // NCORES          # rows per core = 512
TILES = RPC // 128         # i-tiles per core per batch = 4
KS = 15                    # spatial contraction: 3 blocks of (3 coords + 2 norm)
KF = 3 * (D + 2)           # feature contraction: 3 blocks of (32 coords + 2 norm)
KFH = KF // 2              # feature rows are split 51+51 at SBUF partitions 0/64
                           # so the two halves run on disjoint PE row-groups

FP = mybir.dt.float32
F16 = mybir.dt.float16
BF = mybir.dt.bfloat16
AX = mybir.AxisListType
OP = mybir.AluOpType
AF = mybir.ActivationFunctionType

LAST_RESULT = None         # test harness introspection


def _fix_walrus_incompat(nc):
    """This container's walrus codegen fits exactly ONE sync-wait per engine
    instruction struct (Tile's scheduler freely emits several) and rejects the
    EVENT_SEMAPHORE_RANGE_CLEAR raw-ISA instruction Tile emits at context
    exit. Rewrite: (a) every multi-wait instruction becomes (n-1) same-engine
    EventSemaphore waits followed by the instruction with the final wait;
    (b) the range-clear becomes one sem-wr-imm(0) EventSemaphore per sem."""
    import re

    from bass_rust import SyncInfo, SyncUpdate

    fn = nc.m.functions[0]
    originals = [(blk, list(blk.instructions)) for blk in fn.blocks]
    rebuilt = []
    for blk, insts in originals:
        out = []
        for inst in insts:
            tname = type(inst).__name__
            si = inst.sync_info
            if tname == "InstISA" and "EVENT_SEMAPHORE_RANGE_CLEAR" in inst.concise():
                m = re.search(r"range_first=(\d+) range_last=(\d+)", inst.concise())
                first, last = int(m.group(1)), int(m.group(2))
                for sem in range(first, last + 1):
                    ev = mybir.InstEventSemaphore(
                        name=nc.get_next_instruction_name(),
                        engine=inst.engine,
                        sync_info=SyncInfo(
                            on_wait=list(si.on_wait) if si and sem == first else [],
                            on_update=[
                                SyncUpdate(
                                    sync_type="semaphore",
                                    id=sem,
                                    ant_name=f"semclear_{sem}",
                                    update_mode="sem-wr-imm",
                                    update_value=0,
                                    update_reg=None,
                                )
                            ],
                        ),
                    )
                    nc.register_instruction(ev, overwrite=True)
                    out.append(ev)
                continue
            if si is not None and len(si.on_wait) > 1:
                waits = list(si.on_wait)
                for w in waits[:-1]:
                    ev = mybir.InstEventSemaphore(
                        name=nc.get_next_instruction_name(),
                        engine=inst.engine,
                        sync_info=SyncInfo(on_wait=[w], on_update=[]),
                    )
                    nc.register_instruction(ev, overwrite=True)
                    out.append(ev)
                inst.sync_info = SyncInfo(
                    on_wait=[waits[-1]], on_update=list(si.on_update)
                )
            out.append(inst)
        rebuilt.append((blk, out))
    for blk, out in rebuilt:
        blk.instructions[:] = out


def _build_nc(rowsplit=True, ts4=True):
    nc = bass.Bass()

    # lhsT block (cols 0:RPC) FIRST, then rhs (cols RPC:RPC+N) so the first
    # LDWEIGHTS only waits for a small leading DMA chunk.
    spat_comb = nc.dram_tensor("spat_comb", [B, KS, RPC + N], F16, kind="ExternalInput")
    feat_comb = nc.dram_tensor("feat_comb", [B, KF, RPC + N], F16, kind="ExternalInput")
    # biases + weights packed partition-major: smalls[p, tensor*B*TILES + b*TILES + t]
    # = value for row t*128+p of batch b.
    smalls = nc.dram_tensor("smalls", [128, 3 * B * TILES], FP, kind="ExternalInput")
    out = nc.dram_tensor("out", [128, B], FP, kind="ExternalOutput")

    with tile.TileContext(nc) as tc:
        with (
            tc.tile_pool(name="const", bufs=1) as cpool,
            tc.tile_pool(name="psum", bufs=2, space="PSUM") as ppool,
            tc.tile_pool(name="ebuf", bufs=6) as epool,
            tc.tile_pool(name="junk", bufs=3) as jpool,
            tc.tile_pool(name="small", bufs=8) as spool,
            tc.tile_pool(name="spart", bufs=8) as sppool,
            tc.tile_pool(name="accs", bufs=1) as apool,
        ):
            # --- constants / inputs ---
            sm = cpool.tile([128, 3 * B * TILES], FP, tag="smalls")
            nc.sync.dma_start(sm[:], smalls[:])
            bx = [sm[:, b * TILES : (b + 1) * TILES] for b in range(B)]
            bf = [sm[:, (B + b) * TILES : (B + b + 1) * TILES] for b in range(B)]
            wt = [sm[:, (2 * B + b) * TILES : (2 * B + b + 1) * TILES] for b in range(B)]

            # dummy 8-col exp: forces the ACT exp table load during DMA wait
            dum = cpool.tile([128, 8], FP, tag="dum")
            nc.gpsimd.memset(dum[:], 0.0)
            dj = cpool.tile([128, 8], FP, tag="dj")
            nc.scalar.activation(dj[:], dum[:], AF.Exp)

            # whole-tensor transfers: contiguous DRAM rows pack into few
            # large descriptors (column-sliced 2D DMAs cost ~1 descriptor per
            # partition row and take 2-3us just to issue)
            scomb, fcomb = [], []
            for b in range(B):
                st = cpool.tile([KS, RPC + N], F16, tag=f"s{b}")
                nc.sync.dma_start(st[:], spat_comb[b])
                ft = cpool.tile([128, RPC + N], F16, tag=f"f{b}")
                if rowsplit:
                    for h, p0 in ((0, 0), (1, 64)):
                        nc.sync.dma_start(
                            ft[p0 : p0 + KFH, :],
                            feat_comb[b][h * KFH : (h + 1) * KFH, :],
                        )
                else:
                    nc.sync.dma_start(ft[0:KF, :], feat_comb[b][:, :])
                scomb.append(st)
                fcomb.append(ft)

            # PE warmup during the DMA fill (HAM p-state nudge; harmless if cold)
            n_warm = int(os.environ.get("DFL_WARMUP", "0"))
            if n_warm:
                wsrc = cpool.tile([128, 512], BF, tag="warm")
                nc.gpsimd.memset(wsrc[:], 1.0)
                for _ in range(n_warm):
                    pw = ppool.tile([128, 2048], FP, tag="ps")
                    nc.tensor.matmul(
                        pw[:, 0:512], wsrc[:, 0:128], wsrc[:], start=True, stop=True
                    )

            outsb = apool.tile([128, B], FP, tag="outsb")

            for b in range(B):
                qv = apool.tile([128, TILES], FP, tag=f"qv{b}")
                r1 = apool.tile([128, TILES], FP, tag=f"r1{b}")
                st_b, ft_b = scomb[b], fcomb[b]
                units = [None] * TILES

                def emit_field(t, is_feat):
                    u = units[t]
                    ct = ft_b if is_feat else st_b
                    bias_t = bf[b] if is_feat else bx[b]
                    ebuf = u["e2"] if is_feat else u["e1"]
                    for half in range(2):
                        col0 = half * 2048
                        ps = ppool.tile([128, 2048], FP, tag="ps")
                        for k in range(4):
                            c0 = RPC + col0 + k * 512
                            pseg = ps[:, k * 512 : (k + 1) * 512]
                            if is_feat and rowsplit:
                                for h, p0 in ((0, 0), (1, 64)):
                                    nc.tensor.matmul(
                                        pseg,
                                        ct[p0 : p0 + KFH, t * 128 : (t + 1) * 128],
                                        ct[p0 : p0 + KFH, c0 : c0 + 512],
                                        start=(h == 0),
                                        stop=(h == 1),
                                    )
                            elif is_feat:
                                nc.tensor.matmul(
                                    pseg,
                                    ct[0:KF, t * 128 : (t + 1) * 128],
                                    ct[0:KF, c0 : c0 + 512],
                                    start=True,
                                    stop=True,
                                )
                            else:
                                nc.tensor.matmul(
                                    pseg,
                                    ct[:, t * 128 : (t + 1) * 128],
                                    ct[:, c0 : c0 + 512],
                                    start=True,
                                    stop=True,
                                )
                        nc.scalar.activation(
                            ebuf[:, col0 : col0 + 2048],
                            ps[:],
                            AF.Exp,
                            bias=bias_t[:, t : t + 1],
                            accum_out=u["spart"][:, 2 * int(is_feat) + half :
                                                 2 * int(is_feat) + half + 1],
                        )

                def emit_chain(t):
                    u = units[t]
                    e1, e2, spart = u["e1"], u["e2"], u["spart"]
                    ss = spool.tile([128, 2], FP, tag="ss")
                    nc.vector.tensor_reduce(
                        ss[:],
                        spart[:].rearrange("p (m c) -> p m c", c=2),
                        axis=AX.X,
                        op=OP.add,
                    )
                    nc.vector.reciprocal(r1[:, t : t + 1], ss[:, 0:1])
                    rr2 = spool.tile([128, 1], FP, tag="rr2")
                    nc.vector.reciprocal(rr2[:], ss[:, 1:2])
                    cc = spool.tile([128, 1], FP, tag="cc")
                    nc.vector.tensor_tensor(cc[:], ss[:, 0:1], rr2[:], op=OP.mult)
                    junk = jpool.tile([128, N], BF, tag="junk")
                    if ts4:
                        nc.vector.tensor_scalar(
                            junk[:], e2[:], cc[:], None, op0=OP.mult
                        )
                        nc.vector.tensor_tensor(e2[:], junk[:], e1[:], op=OP.subtract)
                    else:
                        nc.vector.scalar_tensor_tensor(
                            e2[:], e2[:], cc[:], e1[:], op0=OP.mult, op1=OP.subtract
                        )
                    nc.vector.scalar_tensor_tensor(
                        junk[:],
                        e2[:],
                        1.0,
                        e2[:],
                        op0=OP.mult,
                        op1=OP.mult,
                        accum_out=qv[:, t : t + 1],
                    )

                # spatial chunks of the first two units run first (they only
                # need the small early spatial DMA); feature chunks weave in
                # once the big feature transfer has landed, so the ACT stream
                # never stalls on the feature DMA descriptor chain.
                order = [(0, 0), (1, 0), (0, 1), (2, 0), (1, 1), (3, 0), (2, 1), (3, 1)]
                for t, is_feat in order:
                    if units[t] is None:
                        e1 = epool.tile([128, N], BF, tag="e1")
                        e2 = epool.tile([128, N], BF, tag="e2")
                        spart = sppool.tile([128, 4], FP, tag="spart")
                        units[t] = {"e1": e1, "e2": e2, "spart": spart}
                    emit_field(t, bool(is_feat))
                    if is_feat:
                        emit_chain(t)

                # loss rows = qv / s1^2, weighted, reduced over the 4 i-tiles
                i2 = spool.tile([128, TILES], FP, tag="i2")
                nc.vector.tensor_tensor(i2[:], r1[:], r1[:], op=OP.mult)
                q2 = spool.tile([128, TILES], FP, tag="q2")
                nc.vector.tensor_tensor(q2[:], qv[:], i2[:], op=OP.mult)
                lw = spool.tile([128, TILES], FP, tag="lw")
                nc.vector.tensor_tensor(lw[:], q2[:], wt[b], op=OP.mult)
                nc.vector.tensor_reduce(
                    outsb[:, b : b + 1], lw[:], axis=AX.X, op=OP.add
                )
                nc.sync.dma_start(out[:, b : b + 1], outsb[:, b : b + 1])

    _fix_walrus_incompat(nc)
    return nc


_NC_CACHE = {}


def _get_nc():
    rowsplit = os.environ.get("DFL_ROWSPLIT", "") == "1"
    ts4 = os.environ.get("DFL_NO_TS4", "") != "1"
    key = (rowsplit, ts4)
    if key not in _NC_CACHE:
        _NC_CACHE[key] = _build_nc(rowsplit, ts4)
    return _NC_CACHE[key]


def _hi_lo_blocks(y, nrm, x, krows):
    """Stacked compensated-product operands.

    y: [B, C, N] rhs coord rows (full precision), nrm: [B, N] rhs norm row,
    x: [B, C, RPC] lhsT coord rows. krows = C + 2 block height.
    Returns (rhs [B, 3*krows, N], lhsT [B, 3*krows, RPC]) fp16 so that
    lhsT.T @ rhs = x.T @ y + nrm  with ~22-bit effective mantissa.
    """
    nb = y.shape[0]
    ncols = y.shape[2]
    nlhs = x.shape[2]
    C = y.shape[1]
    yh = y.astype(np.float16)
    yl = (y - yh.astype(np.float64)).astype(np.float16)
    nh = nrm.astype(np.float16)
    nl = (nrm - nh.astype(np.float64)).astype(np.float16)
    n2 = (nrm - nh.astype(np.float64) - nl.astype(np.float64)).astype(np.float16)
    xh = x.astype(np.float16)
    xl = (x - xh.astype(np.float64)).astype(np.float16)

    hi_r = np.zeros((nb, krows, ncols), np.float16)
    lo_r = np.zeros((nb, krows, ncols), np.float16)
    hi_l = np.zeros((nb, krows, nlhs), np.float16)
    lo_l = np.zeros((nb, krows, nlhs), np.float16)
    hi_r[:, :C] = yh
    hi_r[:, C] = nh
    hi_r[:, C + 1] = n2
    lo_r[:, :C] = yl
    lo_r[:, C] = nl
    hi_l[:, :C] = xh
    hi_l[:, C] = 1.0
    lo_l[:, :C] = xl
    lo_l[:, C + 1] = 1.0
    rhs = np.concatenate([hi_r, lo_r, hi_r], axis=1)
    lhsT = np.concatenate([hi_l, hi_l, lo_l], axis=1)
    return rhs, lhsT


def _prep_inputs(points, pointfea1, pointfea2, weights):
    """Host-side sharding + operand layout. Returns per-core input maps."""
    s2inv = np.float64(1.0) / (SIGMA * SIGMA)
    x = points.astype(np.float64)        # [B, N, 3]
    f1 = pointfea1.astype(np.float64)    # [B, N, D]
    f2 = pointfea2.astype(np.float64)
    w = weights.astype(np.float32)

    xT = np.swapaxes(x, 1, 2)            # [B, 3, N]
    f1T = np.swapaxes(f1, 1, 2)          # [B, D, N]
    f2T = np.swapaxes(f2, 1, 2)

    xn = np.sum(x * x, axis=2)           # [B, N]
    f1n = np.sum(f1 * f1, axis=2)
    f2n = np.sum(f2 * f2, axis=2)

    # rhs operands are shared across cores; build once
    spat_rhs_all = None
    feat_rhs_all = None

    in_maps = []
    for c in range(NCORES):
        sl = slice(c * RPC, (c + 1) * RPC)
        srhs, slhs = _hi_lo_blocks(
            2.0 * s2inv * xT, -s2inv * xn, xT[:, :, sl], 5
        )
        frhs, flhs = _hi_lo_blocks(2.0 * f2T, -f2n, f1T[:, :, sl], D + 2)
        if spat_rhs_all is None:
            spat_rhs_all = srhs
            feat_rhs_all = frhs
        spat_comb = np.concatenate([slhs, spat_rhs_all], axis=2)
        feat_comb = np.concatenate([flhs, feat_rhs_all], axis=2)

        smalls = np.empty((128, 3 * B * TILES), np.float32)
        for b in range(B):
            bxv = (-s2inv * xn[b, sl]).astype(np.float32).reshape(TILES, 128)
            bfv = (-f1n[b, sl]).astype(np.float32).reshape(TILES, 128)
            wv = w[b, sl].reshape(TILES, 128)
            smalls[:, b * TILES : (b + 1) * TILES] = bxv.T
            smalls[:, (B + b) * TILES : (B + b + 1) * TILES] = bfv.T
            smalls[:, (2 * B + b) * TILES : (2 * B + b + 1) * TILES] = wv.T
        in_maps.append(
            {"spat_comb": spat_comb, "feat_comb": feat_comb, "smalls": smalls}
        )
    return in_maps


def kernel(points, pointfea1, pointfea2, weights):
    global LAST_RESULT
    nc = _get_nc()
    in_maps = _prep_inputs(points, pointfea1, pointfea2, weights)
    res = run_bass_kernel_spmd(nc, in_maps, core_ids=list(range(NCORES)))
    LAST_RESULT = res
    total = np.zeros(B, np.float64)
    for m in res.results:
        total += m["out"].astype(np.float64).sum(axis=0)
    return total.astype(np.float32)
